# revision 9
# baseline (speedup 1.0000x reference)
"""Trainium2 Bass kernel for the heap-ancestor MLP — half-space formulation.

Math (validated in sim2.py): all three layers collapse onto the 2048-wide
"node-pair" half space, because out[2a] == out[2a+1] for every layer (the
heap gather j>>k maps both children to the same ancestors for k>=1, and the
k=0 input h[2a]==h[2a+1] too).  Storage convention: column c of a half-space
array holds the value of node-pair (2c, 2c+1) for c>=1, and node 1's value
at c=0 (node 0 is handled host-side — its L2 input h1[0] is 0, making its
whole column a host-computable constant).

  L1: H = W1s.T @ SX, one K=24 stacked matmul per 512-col tile; SX (host) is
      the masked stack [x[p>>m]]_m.  R1 = relu(H + Bc1seg) — per-level
      cumulative bias: tile 0 gets it via a K=12 one-hot matmul (levels mix
      inside the tile), tiles 1-3 are level-uniform so the bias rides the
      activation-copy's per-partition bias operand.
  L2: OH2 = sum_k W2_k R1[maps2048_k] — masked k-step matmul accumulation on
      2048 cols (W2_12 drops out: it only ever multiplies h1[0]=0).
      RH2 = relu(OH2 + Bc2seg), same bias scheme.
  L3: U0 = sum_k W3_k RH2[maps2048_k]  [2, 2048] f32; level-dependent bias
      and the h2[0]-ancestor constants are added on the HOST, which also
      pair-expands to the full 4096 output.

Scheduling: batches are processed in PAIRS — batch b's data lives in SBUF
rows 0-63 (its matmuls use PE row-group 0), batch b+1 in rows 64-127
(row-group 64).  Each k-round issues up to 8 matmuls over the 4 PE
quadrants, keeping the array ~4-way occupied even as tiles drop out at
their kmax.  Outputs whose PSUM col-half differs from the batch's row-half
are duplicated into place with SBUF-SBUF DMAs.

Sharding: pure data parallelism over batch (256 -> 32/core x 8 cores).
"""

import numpy as np

import concourse.bass as bass
from concourse import bacc
import concourse.mybir as mybir
import concourse.tile as tile
from concourse.bass_utils import run_bass_kernel_spmd

UNITS = 4096
HUNITS = 2048
DEPTH = 13
JT = 512
NCORES = 8
BATCH = 256

F32 = mybir.dt.float32
BF16 = mybir.dt.bfloat16

# packed bf16 constant layout [128, _CW]; rows 64+ duplicate rows 0-63
_CREG = {}
_c = 0
for _name, _w in [("w2t", 12 * 64), ("w3t", 12 * 2), ("w1s", 64),
                  ("bb1", 64), ("oh1", JT), ("bb2", 64), ("oh2", JT)]:
    _CREG[_name] = (_c, _c + _w)
    _c += _w
_CW = _c


def _kmax_h(t: int) -> int:
    if t == 0:
        return 9
    return int(np.floor(np.log2(JT * t))) + 1


def _bcast(ap, n_dist: int, rep: int):
    return ap.unsqueeze(2).broadcast_to([ap.shape[0], n_dist, rep])


def _emit_heap_mms(nc, entries, w_of, m, kw, t0_open, after_k0=None, t0_close=True):
    """k-step heap matmuls.  entries: (ps, r, c, t, h) x8 (two batches x 4
    half-space tiles).  t0_open: tile-0's k=0 opens its accumulation group
    (True when no bias matmul was emitted first).  Tile 0 always closes its
    own group (stop on its last matmul)."""
    kmaxes = [_kmax_h(t) for _, _, _, t, _ in entries]
    for k in range(0, max(kmaxes) + 1):
        deferred = []
        if k == 1 and after_k0 is not None:
            after_k0()
        for (ps, r, c, t, h), kmax in zip(entries, kmaxes):
            if k > kmax:
                continue
            c0 = t * JT
            hs = h[r : r + kw, :]
            om = ps[c : c + m, :]
            last = k == kmax
            if k == 0:
                nc.tensor.matmul(
                    om, w_of(r, k), hs[:, c0 : c0 + JT],
                    start=(t != 0) or t0_open, stop=last and t != 0,
                    tile_position=(r, c), skip_group_check=True,
                )
            elif t == 0:
                half = 1 << (k - 1)
                blk = 1 << k
                nc.tensor.matmul(
                    ps[c : c + m, half:blk],
                    w_of(r, k),
                    hs[:, 0:1].broadcast_to([kw, half]),
                    start=False, stop=last and t0_close and blk >= JT,
                    tile_position=(r, c), skip_group_check=True,
                )
                if blk < JT:
                    nd = (JT >> k) - 1
                    deferred.append((ps, r, c, m, kw, hs, nd, blk, last and t0_close, k))
            elif (JT >> k) >= 1:
                nd = JT >> k
                a0 = c0 >> k
                nc.tensor.matmul(
                    om, w_of(r, k), _bcast(hs[:, a0 : a0 + nd], nd, 1 << k),
                    start=False, stop=last,
                    tile_position=(r, c), skip_group_check=True,
                )
            else:
                a = c0 >> k
                nc.tensor.matmul(
                    om, w_of(r, k), hs[:, a : a + 1].broadcast_to([kw, JT]),
                    start=False, stop=last,
                    tile_position=(r, c), skip_group_check=True,
                )
        for ps, r, c, m2, kw2, hs2, nd, blk, last2, kk in deferred:
            nc.tensor.matmul(
                ps[c : c + m2, blk:JT],
                w_of(r, kk),
                _bcast(hs2[:, 1 : 1 + nd], nd, blk),
                start=False, stop=last2,
                tile_position=(r, c), skip_group_check=True,
            )


# Per batch-pair tile->quad-slot assignment.  Batch 0 of the pair streams
# from SBUF rows 0-63 (r=0), batch 1 from rows 64-127 (r=64).  Output col
# position alternates so each quad generation covers all 4 PE quadrants.
# (bi, t, r, c): bi = index within pair.
_SLOTS = [
    # quad A: tiles 0,1 of both batches (emission alternates row groups so
    # each LDWEIGHTS can overlap the other row-half's in-flight stream)
    [(0, 0, 0, 0), (1, 0, 64, 0), (0, 1, 0, 64), (1, 1, 64, 64)],
    # quad B: tiles 2,3
    [(0, 2, 0, 0), (1, 2, 64, 0), (0, 3, 0, 64), (1, 3, 64, 64)],
]
# L3 (m=2): all 8 tiles in ONE generation — 4 col strips x 2 row groups
# gives 8 disjoint PE rectangles (8 concurrent streams, 8 PSUM banks).
_SLOTS3 = [[
    (0, 0, 0, 0), (1, 0, 64, 0), (0, 1, 0, 32), (1, 1, 64, 32),
    (0, 2, 0, 64), (1, 2, 64, 64), (0, 3, 0, 96), (1, 3, 64, 96),
]]


def build_program(nb: int) -> bass.Bass:
    nc = bacc.Bacc()
    sxd = nc.declare_dram_parameter("sxd", [nb, 24, HUNITS], BF16, isOutput=False)
    cst = nc.declare_dram_parameter("cst", [128, _CW], BF16, isOutput=False)
    cstf = nc.declare_dram_parameter("cstf", [128, 4], F32, isOutput=False)
    outh = nc.declare_dram_parameter("outh", [nb, 2, HUNITS], F32, isOutput=True)

    relu = mybir.ActivationFunctionType.Relu
    add = mybir.AluOpType.add
    amax = mybir.AluOpType.max

    with tile.TileContext(nc) as tc:
        with (
            tc.tile_pool(name="const", bufs=1) as cp,
            tc.tile_pool(name="sx", bufs=3) as sxp,
            tc.tile_pool(name="r1", bufs=3) as r1p,
            tc.tile_pool(name="rh2", bufs=3) as r2p,
            tc.tile_pool(name="st", bufs=2) as stp,
            tc.tile_pool(name="ps", bufs=8, space="PSUM") as psp,
        ):
            cst_sb = cp.tile([128, _CW], BF16)
            nc.sync.dma_start(out=cst_sb[:], in_=cst[:, :])
            cstf_sb = cp.tile([128, 4], F32)
            nc.sync.dma_start(out=cstf_sb[:], in_=cstf[:, :])

            def _reg(name, rows):
                lo, hi = _CREG[name]

                def f(r):
                    return cst_sb[r : r + rows, lo:hi]

                return f

            w1s = _reg("w1s", 24)
            w2t = _reg("w2t", 64)
            w3t = _reg("w3t", 64)
            bb1 = _reg("bb1", 12)
            oh1 = _reg("oh1", 12)
            bb2 = _reg("bb2", 13)
            oh2 = _reg("oh2", 13)

            def w2_of(r, k):
                return w2t(r)[:, 64 * k : 64 * (k + 1)]

            def w3_of(r, k):
                return w3t(r)[:, 2 * k : 2 * (k + 1)]

            def bv(i, p):
                # bias vector APs (f32), at partition base p (0 or 64)
                return cstf_sb[p : p + 64, i : i + 1]

            def relu_copies(nc, quads, dst, bias_idx):
                """PSUM->SBUF relu copies for one layer of a batch pair.
                dst: single [128, 2048] tile, rows 0-63 = batch 0 of the
                pair, 64-127 = batch 1.  Copies go straight to the batch's
                half (cross-partition-base PSUM reads are legal)."""
                eng = [0, 1, 0, 1, 1, 0, 1, 0]
                for i, (ps, bi, t, r, c) in enumerate(quads):
                    want = 64 * bi
                    cs = slice(t * JT, (t + 1) * JT)
                    b_idx = bias_idx[t]
                    src = ps[c : c + 64, :]
                    out = dst[want : want + 64, cs]
                    if b_idx is None:
                        if eng[i] == 0:
                            nc.scalar.activation(out, src, relu)
                        else:
                            nc.vector.tensor_scalar(out, src, 0.0, None, amax)
                    else:
                        if eng[i] == 0:
                            nc.scalar.activation(out, src, relu, bias=bv(b_idx, c))
                        else:
                            nc.vector.tensor_scalar(out, src, bv(b_idx, c), 0.0,
                                                    add, amax)

            def emit_l1(bp):
                b0 = 2 * bp
                sx_sb = sxp.tile([128, HUNITS], BF16, name="sx_sb")
                nc.sync.dma_start(out=sx_sb[0:24, :], in_=sxd[b0, :, :])
                nc.sync.dma_start(out=sx_sb[64:88, :], in_=sxd[b0 + 1, :, :])
                r1 = r1p.tile([128, HUNITS], BF16, name="r1")
                for grp in _SLOTS:
                    quads = []
                    for bi, t, r, c in grp:
                        pq = psp.tile([128, JT], F32, tag="ps", name="pq")
                        quads.append((pq, bi, t, r, c))
                    for ps, bi, t, r, c in quads:
                        if t == 0:
                            nc.tensor.matmul(
                                ps[c : c + 64, :], bb1(r), oh1(r),
                                start=True, stop=False,
                                tile_position=(r, c), skip_group_check=True,
                            )
                    for ps, bi, t, r, c in quads:
                        nc.tensor.matmul(
                            ps[c : c + 64, :], w1s(r),
                            sx_sb[r : r + 24, t * JT : (t + 1) * JT],
                            start=t != 0, stop=True,
                            tile_position=(r, c), skip_group_check=True,
                        )
                    relu_copies(nc, quads, r1, [None, 0, 1, 1])
                return r1

            def emit_l2(bp, r1):
                rh2 = r2p.tile([128, HUNITS], BF16, name="rh2")
                for grp in _SLOTS:
                    quads = []
                    for bi, t, r, c in grp:
                        pq = psp.tile([128, JT], F32, tag="ps", name="pq")
                        quads.append((pq, bi, t, r, c))
                    entries = [(ps, r, c, t, r1[:]) for ps, bi, t, r, c in quads]
                    _emit_heap_mms(nc, entries, w2_of, 64, 64, t0_open=True,
                                   t0_close=False)
                    for ps, bi, t, r, c in quads:
                        if t == 0:
                            nc.tensor.matmul(
                                ps[c : c + 64, :], bb2(r), oh2(r),
                                start=False, stop=True,
                                tile_position=(r, c), skip_group_check=True,
                            )
                    relu_copies(nc, quads, rh2, [None, 2, 3, 3])
                return rh2

            def emit_l3(bp, rh2):
                b0 = 2 * bp
                st = stp.tile([128, 2 * HUNITS], F32, name="st")
                for grp in _SLOTS3:
                    quads = []
                    for bi, t, r, c in grp:
                        pq = psp.tile([128, JT], F32, tag="ps", name="pq")
                        quads.append((pq, bi, t, r, c))
                    entries = [(ps, r, c, t, rh2[:]) for ps, bi, t, r, c in quads]
                    _emit_heap_mms(nc, entries, w3_of, 2, 64, t0_open=True)
                    for ps, bi, t, r, c in quads:
                        o = bi * HUNITS + t * JT
                        nc.any.tensor_copy(st[c : c + 2, o : o + JT], ps[c : c + 2, :])
                    for ps, bi, t, r, c in quads:
                        o = bi * HUNITS + t * JT
                        nc.sync.dma_start(
                            out=outh[b0 + bi, :, t * JT : (t + 1) * JT],
                            in_=st[c : c + 2, o : o + JT],
                        )

            # software pipeline over batch pairs: L2(p) / L1(p+1) / L3(p) —
            # each block's inputs are produced by copies that overlap the
            # previous block's matmuls, so the PE never hits a layer barrier.
            npairs = nb // 2
            r1_cur = emit_l1(0)
            rh2_prev = None
            for p in range(npairs):
                rh2_cur = emit_l2(p, r1_cur)
                if p + 1 < npairs:
                    r1_cur = emit_l1(p + 1)
                if rh2_prev is not None:
                    emit_l3(p - 1, rh2_prev)
                rh2_prev = rh2_cur
            emit_l3(npairs - 1, rh2_prev)
    nc.compile()
    return nc


def _prep_host(x, W1, b1, W2, b2, W3, b3):
    npbf = np.dtype(mybir.dt.np(BF16))
    lam = np.full(HUNITS, -1, np.int64)
    lam[1:] = np.floor(np.log2(np.arange(1, HUNITS))).astype(np.int64)

    B = x.shape[0]
    xt = np.ascontiguousarray(x.transpose(0, 2, 1))  # [B, 2, 4096]
    SX = np.zeros((B, 24, HUNITS), np.float32)
    for m in range(12):
        nd = HUNITS >> m
        rep = np.repeat(xt[:, :, :nd], 1 << m, axis=2)[:, :, :HUNITS]
        lo = 1 << (m - 1) if m >= 1 else 0
        if lo:
            rep[:, :, :lo] = 0.0
        SX[:, 2 * m : 2 * m + 2, :] = rep

    cum1 = np.cumsum(b1[:12], axis=0)
    cum2 = np.cumsum(b2, axis=0)
    w1s = np.zeros((24, 64), np.float32)
    for m in range(12):
        w1s[2 * m : 2 * m + 2, :] = W1[m].T
    w2t = W2[:12].transpose(2, 0, 1).reshape(64, 12 * 64)
    w3t = W3[:12].transpose(2, 0, 1).reshape(64, 12 * 2)

    L1lev = np.zeros(JT, np.int64)
    L1lev[1:] = lam[1:JT] + 1
    oh1 = np.zeros((12, JT), np.float32)
    oh1[L1lev, np.arange(JT)] = 1.0
    bb1 = cum1

    seg = np.zeros(JT, np.int64)
    seg[1:] = lam[1:JT] + 1
    oh2 = np.zeros((13, JT), np.float32)
    oh2[seg, np.arange(JT)] = 1.0
    bb2 = np.zeros((13, 64), np.float32)
    bb2[0] = cum2[1]
    for s in range(12):
        bb2[s + 1] = cum2[min(s + 2, 12)]

    cstm = np.zeros((128, _CW), np.float32)
    for name, a in [("w2t", w2t), ("w3t", w3t), ("w1s", w1s),
                    ("bb1", bb1), ("oh1", oh1), ("bb2", bb2), ("oh2", oh2)]:
        lo, hi = _CREG[name]
        cstm[: a.shape[0], lo:hi] = a
        cstm[64 : 64 + a.shape[0], lo:hi] = a

    cstfm = np.zeros((128, 4), np.float32)
    for half in (0, 64):
        cstfm[half : half + 64, 0] = cum1[10]
        cstfm[half : half + 64, 1] = cum1[11]
        cstfm[half : half + 64, 2] = cum2[11]
        cstfm[half : half + 64, 3] = cum2[12]

    common = {"cst": cstm.astype(npbf), "cstf": cstfm}
    nb = B // NCORES
    in_maps = []
    for c in range(NCORES):
        m = dict(common)
        m["sxd"] = SX[c * nb : (c + 1) * nb].astype(npbf)
        in_maps.append(m)

    rh2_0 = np.maximum(b2[0], 0.0)
    cum3 = np.cumsum(b3, axis=0)
    hb = np.zeros((HUNITS, 2), np.float32)
    hb[0] = cum3[1] + W3[1] @ rh2_0
    kk = np.minimum(lam[1:] + 2, 12)
    hb[1:] = cum3[kk] + np.einsum("nij,j->ni", W3[kk], rh2_0)
    out0 = W3[0] @ rh2_0 + b3[0]
    return in_maps, hb, out0


def kernel(x, W1, b1, W2, b2, W3, b3, _trace=False, _mode=None):
    nb = x.shape[0] // NCORES
    nc = build_program(nb)
    in_maps, hb, out0 = _prep_host(x, W1, b1, W2, b2, W3, b3)
    res = run_bass_kernel_spmd(
        nc, in_maps, core_ids=list(range(NCORES)), trace=_trace
    )
    outs = []
    for r in res.results:
        u0 = r["outh"].astype(np.float32)        # [nb, 2, 2048]
        u0 = u0 + hb.T[None]
        full = np.empty((u0.shape[0], UNITS, 2), np.float32)
        full[:, 0, :] = out0[None]
        full[:, 1, :] = u0[:, :, 0]
        full[:, 2:, :] = np.repeat(u0[:, :, 1:], 2, axis=2).transpose(0, 2, 1)
        outs.append(full)
    out = np.concatenate(outs, 0)
    if _trace:
        kernel.last_exec_time_ns = res.exec_time_ns
        kernel.last_results = res
    return out


kernel.last_exec_time_ns = None
kernel.last_results = None
MODE = "bf16-halfspace-pair"


# revision 10
# speedup vs baseline: 1.0450x; 1.0450x over previous
"""Trainium2 Bass kernel for the heap-ancestor MLP — half-space formulation.

Math (validated in sim2.py): all three layers collapse onto the 2048-wide
"node-pair" half space, because out[2a] == out[2a+1] for every layer (the
heap gather j>>k maps both children to the same ancestors for k>=1, and the
k=0 input h[2a]==h[2a+1] too).  Storage convention: column c of a half-space
array holds the value of node-pair (2c, 2c+1) for c>=1, and node 1's value
at c=0 (node 0 is handled host-side — its L2 input h1[0] is 0, making its
whole column a host-computable constant).

  L1: H = W1s.T @ SX, one K=24 stacked matmul per 512-col tile; SX (host) is
      the masked stack [x[p>>m]]_m.  R1 = relu(H + Bc1seg) — per-level
      cumulative bias: tile 0 gets it via a K=12 one-hot matmul (levels mix
      inside the tile), tiles 1-3 are level-uniform so the bias rides the
      activation-copy's per-partition bias operand.
  L2: OH2 = sum_k W2_k R1[maps2048_k] — masked k-step matmul accumulation on
      2048 cols (W2_12 drops out: it only ever multiplies h1[0]=0).
      RH2 = relu(OH2 + Bc2seg), same bias scheme.
  L3: U0 = sum_k W3_k RH2[maps2048_k]  [2, 2048] f32; level-dependent bias
      and the h2[0]-ancestor constants are added on the HOST, which also
      pair-expands to the full 4096 output.

Scheduling: batches are processed in PAIRS — batch b's data lives in SBUF
rows 0-63 (its matmuls use PE row-group 0), batch b+1 in rows 64-127
(row-group 64).  Each k-round issues up to 8 matmuls over the 4 PE
quadrants, keeping the array ~4-way occupied even as tiles drop out at
their kmax.  Outputs whose PSUM col-half differs from the batch's row-half
are duplicated into place with SBUF-SBUF DMAs.

Sharding: pure data parallelism over batch (256 -> 32/core x 8 cores).
"""

import numpy as np

import concourse.bass as bass
from concourse import bacc
import concourse.mybir as mybir
import concourse.tile as tile
from concourse.bass_utils import run_bass_kernel_spmd

UNITS = 4096
HUNITS = 2048
DEPTH = 13
JT = 512
NCORES = 8
BATCH = 256

F32 = mybir.dt.float32
BF16 = mybir.dt.bfloat16

# packed bf16 constant layout [128, _CW]; rows 64+ duplicate rows 0-63
_CREG = {}
_c = 0
for _name, _w in [("w2t", 12 * 64), ("w3t", 12 * 2), ("w1s", 64),
                  ("bb1", 64), ("oh1", JT), ("bb2", 64), ("oh2", JT)]:
    _CREG[_name] = (_c, _c + _w)
    _c += _w
_CW = _c


def _kmax_h(t: int) -> int:
    if t == 0:
        return 9
    return int(np.floor(np.log2(JT * t))) + 1


def _bcast(ap, n_dist: int, rep: int):
    return ap.unsqueeze(2).broadcast_to([ap.shape[0], n_dist, rep])


def _emit_heap_mms(nc, entries, w_of, m, kw, t0_open, after_k0=None):
    """k-step heap matmuls.  entries: (ps, r, c, t, h) x8 (two batches x 4
    half-space tiles).  t0_open: tile-0's k=0 opens its accumulation group
    (True when no bias matmul was emitted first).  Tile 0 always closes its
    own group (stop on its last matmul)."""
    kmaxes = [_kmax_h(t) for _, _, _, t, _ in entries]
    for k in range(0, max(kmaxes) + 1):
        deferred = []
        if k == 1 and after_k0 is not None:
            after_k0()
        for (ps, r, c, t, h), kmax in zip(entries, kmaxes):
            if k > kmax:
                continue
            c0 = t * JT
            hs = h[r : r + kw, :]
            om = ps[c : c + m, :]
            last = k == kmax
            if k == 0:
                nc.tensor.matmul(
                    om, w_of(r, k), hs[:, c0 : c0 + JT],
                    start=(t != 0) or t0_open, stop=last and t != 0,
                    tile_position=(r, c), skip_group_check=True,
                )
            elif t == 0:
                half = 1 << (k - 1)
                blk = 1 << k
                nc.tensor.matmul(
                    ps[c : c + m, half:blk],
                    w_of(r, k),
                    hs[:, 0:1].broadcast_to([kw, half]),
                    start=False, stop=last and blk >= JT,
                    tile_position=(r, c), skip_group_check=True,
                )
                if blk < JT:
                    nd = (JT >> k) - 1
                    deferred.append((ps, r, c, m, kw, hs, nd, blk, last, k))
            elif (JT >> k) >= 1:
                nd = JT >> k
                a0 = c0 >> k
                nc.tensor.matmul(
                    om, w_of(r, k), _bcast(hs[:, a0 : a0 + nd], nd, 1 << k),
                    start=False, stop=last,
                    tile_position=(r, c), skip_group_check=True,
                )
            else:
                a = c0 >> k
                nc.tensor.matmul(
                    om, w_of(r, k), hs[:, a : a + 1].broadcast_to([kw, JT]),
                    start=False, stop=last,
                    tile_position=(r, c), skip_group_check=True,
                )
        for ps, r, c, m2, kw2, hs2, nd, blk, last2, kk in deferred:
            nc.tensor.matmul(
                ps[c : c + m2, blk:JT],
                w_of(r, kk),
                _bcast(hs2[:, 1 : 1 + nd], nd, blk),
                start=False, stop=last2,
                tile_position=(r, c), skip_group_check=True,
            )


# Per batch-pair tile->quad-slot assignment.  Batch 0 of the pair streams
# from SBUF rows 0-63 (r=0), batch 1 from rows 64-127 (r=64).  Output col
# position alternates so each quad generation covers all 4 PE quadrants.
# (bi, t, r, c): bi = index within pair.
_SLOTS = [
    # quad A: tiles 0,1 of both batches (emission alternates row groups so
    # each LDWEIGHTS can overlap the other row-half's in-flight stream)
    [(0, 0, 0, 0), (1, 0, 64, 0), (0, 1, 0, 64), (1, 1, 64, 64)],
    # quad B: tiles 2,3
    [(0, 2, 0, 0), (1, 2, 64, 0), (0, 3, 0, 64), (1, 3, 64, 64)],
]
# L3 (m=2): all 8 tiles in ONE generation — 4 col strips x 2 row groups
# gives 8 disjoint PE rectangles (8 concurrent streams, 8 PSUM banks).
_SLOTS3 = [[
    (0, 0, 0, 0), (1, 0, 64, 0), (0, 1, 0, 32), (1, 1, 64, 32),
    (0, 2, 0, 64), (1, 2, 64, 64), (0, 3, 0, 96), (1, 3, 64, 96),
]]


def build_program(nb: int) -> bass.Bass:
    nc = bacc.Bacc()
    sxd = nc.declare_dram_parameter("sxd", [nb, 24, HUNITS], BF16, isOutput=False)
    cst = nc.declare_dram_parameter("cst", [128, _CW], BF16, isOutput=False)
    cstf = nc.declare_dram_parameter("cstf", [128, 4], F32, isOutput=False)
    outh = nc.declare_dram_parameter("outh", [nb, 2, HUNITS], F32, isOutput=True)

    relu = mybir.ActivationFunctionType.Relu
    add = mybir.AluOpType.add
    amax = mybir.AluOpType.max

    with tile.TileContext(nc) as tc:
        with (
            tc.tile_pool(name="const", bufs=1) as cp,
            tc.tile_pool(name="sx", bufs=3) as sxp,
            tc.tile_pool(name="r1", bufs=3) as r1p,
            tc.tile_pool(name="rh2", bufs=3) as r2p,
            tc.tile_pool(name="st", bufs=2) as stp,
            tc.tile_pool(name="ps", bufs=8, space="PSUM") as psp,
        ):
            cst_sb = cp.tile([128, _CW], BF16)
            nc.sync.dma_start(out=cst_sb[:], in_=cst[:, :])
            cstf_sb = cp.tile([128, 4], F32)
            nc.sync.dma_start(out=cstf_sb[:], in_=cstf[:, :])

            def _reg(name, rows):
                lo, hi = _CREG[name]

                def f(r):
                    return cst_sb[r : r + rows, lo:hi]

                return f

            w1s = _reg("w1s", 24)
            w2t = _reg("w2t", 64)
            w3t = _reg("w3t", 64)
            bb1 = _reg("bb1", 12)
            oh1 = _reg("oh1", 12)
            bb2 = _reg("bb2", 13)
            oh2 = _reg("oh2", 13)

            def w2_of(r, k):
                return w2t(r)[:, 64 * k : 64 * (k + 1)]

            def w3_of(r, k):
                return w3t(r)[:, 2 * k : 2 * (k + 1)]

            def bv(i, p):
                # bias vector APs (f32), at partition base p (0 or 64)
                return cstf_sb[p : p + 64, i : i + 1]

            def relu_copies(nc, quads, dst, bias_idx):
                """PSUM->SBUF relu copies for one layer of a batch pair.
                dst: single [128, 2048] tile, rows 0-63 = batch 0 of the
                pair, 64-127 = batch 1.  Copies go straight to the batch's
                half (cross-partition-base PSUM reads are legal)."""
                eng = [0, 1, 0, 1, 1, 0, 1, 0]
                for i, (ps, bi, t, r, c) in enumerate(quads):
                    want = 64 * bi
                    cs = slice(t * JT, (t + 1) * JT)
                    b_idx = bias_idx[t]
                    src = ps[c : c + 64, :]
                    out = dst[want : want + 64, cs]
                    if b_idx is None:
                        if eng[i] == 0:
                            nc.scalar.activation(out, src, relu)
                        else:
                            nc.vector.tensor_scalar(out, src, 0.0, None, amax)
                    else:
                        if eng[i] == 0:
                            nc.scalar.activation(out, src, relu, bias=bv(b_idx, c))
                        else:
                            nc.vector.tensor_scalar(out, src, bv(b_idx, c), 0.0,
                                                    add, amax)

            def emit_l1(bp):
                b0 = 2 * bp
                sx_sb = sxp.tile([128, HUNITS], BF16, name="sx_sb")
                nc.sync.dma_start(out=sx_sb[0:24, :], in_=sxd[b0, :, :])
                nc.sync.dma_start(out=sx_sb[64:88, :], in_=sxd[b0 + 1, :, :])
                r1 = r1p.tile([128, HUNITS], BF16, name="r1")
                for grp in _SLOTS:
                    quads = []
                    for bi, t, r, c in grp:
                        pq = psp.tile([128, JT], F32, tag="ps", name="pq")
                        quads.append((pq, bi, t, r, c))
                    for ps, bi, t, r, c in quads:
                        if t == 0:
                            nc.tensor.matmul(
                                ps[c : c + 64, :], bb1(r), oh1(r),
                                start=True, stop=False,
                                tile_position=(r, c), skip_group_check=True,
                            )
                    for ps, bi, t, r, c in quads:
                        nc.tensor.matmul(
                            ps[c : c + 64, :], w1s(r),
                            sx_sb[r : r + 24, t * JT : (t + 1) * JT],
                            start=t != 0, stop=True,
                            tile_position=(r, c), skip_group_check=True,
                        )
                    relu_copies(nc, quads, r1, [None, 0, 1, 1])
                return r1

            def emit_l2(bp, r1):
                rh2 = r2p.tile([128, HUNITS], BF16, name="rh2")
                for grp in _SLOTS:
                    quads = []
                    for bi, t, r, c in grp:
                        pq = psp.tile([128, JT], F32, tag="ps", name="pq")
                        quads.append((pq, bi, t, r, c))
                    for ps, bi, t, r, c in quads:
                        if t == 0:
                            nc.tensor.matmul(
                                ps[c : c + 64, :], bb2(r), oh2(r),
                                start=True, stop=False,
                                tile_position=(r, c), skip_group_check=True,
                            )
                    entries = [(ps, r, c, t, r1[:]) for ps, bi, t, r, c in quads]
                    _emit_heap_mms(nc, entries, w2_of, 64, 64, t0_open=False)
                    relu_copies(nc, quads, rh2, [None, 2, 3, 3])
                return rh2

            def emit_l3(bp, rh2):
                b0 = 2 * bp
                st = stp.tile([128, 2 * HUNITS], F32, name="st")
                for grp in _SLOTS3:
                    quads = []
                    for bi, t, r, c in grp:
                        pq = psp.tile([128, JT], F32, tag="ps", name="pq")
                        quads.append((pq, bi, t, r, c))
                    entries = [(ps, r, c, t, rh2[:]) for ps, bi, t, r, c in quads]
                    _emit_heap_mms(nc, entries, w3_of, 2, 64, t0_open=True)
                    for ps, bi, t, r, c in quads:
                        o = bi * HUNITS + t * JT
                        nc.any.tensor_copy(st[c : c + 2, o : o + JT], ps[c : c + 2, :])
                    for ps, bi, t, r, c in quads:
                        o = bi * HUNITS + t * JT
                        nc.sync.dma_start(
                            out=outh[b0 + bi, :, t * JT : (t + 1) * JT],
                            in_=st[c : c + 2, o : o + JT],
                        )

            # software pipeline over batch pairs: L2(p) / L1(p+1) / L3(p) —
            # each block's inputs are produced by copies that overlap the
            # previous block's matmuls, so the PE never hits a layer barrier.
            npairs = nb // 2
            r1_cur = emit_l1(0)
            rh2_prev = None
            for p in range(npairs):
                rh2_cur = emit_l2(p, r1_cur)
                if p + 1 < npairs:
                    r1_cur = emit_l1(p + 1)
                if rh2_prev is not None:
                    emit_l3(p - 1, rh2_prev)
                rh2_prev = rh2_cur
            emit_l3(npairs - 1, rh2_prev)
    nc.compile()
    return nc


def _prep_host(x, W1, b1, W2, b2, W3, b3):
    npbf = np.dtype(mybir.dt.np(BF16))
    lam = np.full(HUNITS, -1, np.int64)
    lam[1:] = np.floor(np.log2(np.arange(1, HUNITS))).astype(np.int64)

    B = x.shape[0]
    xt = np.ascontiguousarray(x.transpose(0, 2, 1))  # [B, 2, 4096]
    SX = np.zeros((B, 24, HUNITS), np.float32)
    for m in range(12):
        nd = HUNITS >> m
        rep = np.repeat(xt[:, :, :nd], 1 << m, axis=2)[:, :, :HUNITS]
        lo = 1 << (m - 1) if m >= 1 else 0
        if lo:
            rep[:, :, :lo] = 0.0
        SX[:, 2 * m : 2 * m + 2, :] = rep

    cum1 = np.cumsum(b1[:12], axis=0)
    cum2 = np.cumsum(b2, axis=0)
    w1s = np.zeros((24, 64), np.float32)
    for m in range(12):
        w1s[2 * m : 2 * m + 2, :] = W1[m].T
    w2t = W2[:12].transpose(2, 0, 1).reshape(64, 12 * 64)
    w3t = W3[:12].transpose(2, 0, 1).reshape(64, 12 * 2)

    L1lev = np.zeros(JT, np.int64)
    L1lev[1:] = lam[1:JT] + 1
    oh1 = np.zeros((12, JT), np.float32)
    oh1[L1lev, np.arange(JT)] = 1.0
    bb1 = cum1

    seg = np.zeros(JT, np.int64)
    seg[1:] = lam[1:JT] + 1
    oh2 = np.zeros((13, JT), np.float32)
    oh2[seg, np.arange(JT)] = 1.0
    bb2 = np.zeros((13, 64), np.float32)
    bb2[0] = cum2[1]
    for s in range(12):
        bb2[s + 1] = cum2[min(s + 2, 12)]

    cstm = np.zeros((128, _CW), np.float32)
    for name, a in [("w2t", w2t), ("w3t", w3t), ("w1s", w1s),
                    ("bb1", bb1), ("oh1", oh1), ("bb2", bb2), ("oh2", oh2)]:
        lo, hi = _CREG[name]
        cstm[: a.shape[0], lo:hi] = a
        cstm[64 : 64 + a.shape[0], lo:hi] = a

    cstfm = np.zeros((128, 4), np.float32)
    for half in (0, 64):
        cstfm[half : half + 64, 0] = cum1[10]
        cstfm[half : half + 64, 1] = cum1[11]
        cstfm[half : half + 64, 2] = cum2[11]
        cstfm[half : half + 64, 3] = cum2[12]

    common = {"cst": cstm.astype(npbf), "cstf": cstfm}
    nb = B // NCORES
    in_maps = []
    for c in range(NCORES):
        m = dict(common)
        m["sxd"] = SX[c * nb : (c + 1) * nb].astype(npbf)
        in_maps.append(m)

    rh2_0 = np.maximum(b2[0], 0.0)
    cum3 = np.cumsum(b3, axis=0)
    hb = np.zeros((HUNITS, 2), np.float32)
    hb[0] = cum3[1] + W3[1] @ rh2_0
    kk = np.minimum(lam[1:] + 2, 12)
    hb[1:] = cum3[kk] + np.einsum("nij,j->ni", W3[kk], rh2_0)
    out0 = W3[0] @ rh2_0 + b3[0]
    return in_maps, hb, out0


def kernel(x, W1, b1, W2, b2, W3, b3, _trace=False, _mode=None):
    nb = x.shape[0] // NCORES
    nc = build_program(nb)
    in_maps, hb, out0 = _prep_host(x, W1, b1, W2, b2, W3, b3)
    res = run_bass_kernel_spmd(
        nc, in_maps, core_ids=list(range(NCORES)), trace=_trace
    )
    outs = []
    for r in res.results:
        u0 = r["outh"].astype(np.float32)        # [nb, 2, 2048]
        u0 = u0 + hb.T[None]
        full = np.empty((u0.shape[0], UNITS, 2), np.float32)
        full[:, 0, :] = out0[None]
        full[:, 1, :] = u0[:, :, 0]
        full[:, 2:, :] = np.repeat(u0[:, :, 1:], 2, axis=2).transpose(0, 2, 1)
        outs.append(full)
    out = np.concatenate(outs, 0)
    if _trace:
        kernel.last_exec_time_ns = res.exec_time_ns
        kernel.last_results = res
    return out


kernel.last_exec_time_ns = None
kernel.last_results = None
MODE = "bf16-halfspace-pair"


# revision 12
# speedup vs baseline: 1.0459x; 1.0009x over previous
"""Trainium2 Bass kernel for the heap-ancestor MLP — half-space formulation.

Math (validated in sim2.py): all three layers collapse onto the 2048-wide
"node-pair" half space, because out[2a] == out[2a+1] for every layer (the
heap gather j>>k maps both children to the same ancestors for k>=1, and the
k=0 input h[2a]==h[2a+1] too).  Storage convention: column c of a half-space
array holds the value of node-pair (2c, 2c+1) for c>=1, and node 1's value
at c=0 (node 0 is handled host-side — its L2 input h1[0] is 0, making its
whole column a host-computable constant).

  L1: H = W1s.T @ SX, one K=24 stacked matmul per 512-col tile; SX (host) is
      the masked stack [x[p>>m]]_m.  R1 = relu(H + Bc1seg) — per-level
      cumulative bias: tile 0 gets it via a K=12 one-hot matmul (levels mix
      inside the tile), tiles 1-3 are level-uniform so the bias rides the
      activation-copy's per-partition bias operand.
  L2: OH2 = sum_k W2_k R1[maps2048_k] — masked k-step matmul accumulation on
      2048 cols (W2_12 drops out: it only ever multiplies h1[0]=0).
      RH2 = relu(OH2 + Bc2seg), same bias scheme.
  L3: U0 = sum_k W3_k RH2[maps2048_k]  [2, 2048] f32; level-dependent bias
      and the h2[0]-ancestor constants are added on the HOST, which also
      pair-expands to the full 4096 output.

Scheduling: batches are processed in PAIRS — batch b's data lives in SBUF
rows 0-63 (its matmuls use PE row-group 0), batch b+1 in rows 64-127
(row-group 64).  Each k-round issues up to 8 matmuls over the 4 PE
quadrants, keeping the array ~4-way occupied even as tiles drop out at
their kmax.  Outputs whose PSUM col-half differs from the batch's row-half
are duplicated into place with SBUF-SBUF DMAs.

Sharding: pure data parallelism over batch (256 -> 32/core x 8 cores).
"""

import numpy as np

import concourse.bass as bass
from concourse import bacc
import concourse.mybir as mybir
import concourse.tile as tile
from concourse.bass_utils import run_bass_kernel_spmd

UNITS = 4096
HUNITS = 2048
DEPTH = 13
JT = 512
NCORES = 8
BATCH = 256

F32 = mybir.dt.float32
BF16 = mybir.dt.bfloat16

# packed bf16 constant layout [128, _CW]; rows 64+ duplicate rows 0-63
_CREG = {}
_c = 0
for _name, _w in [("w2t", 12 * 64), ("w3t", 12 * 2), ("w1s", 64),
                  ("bb1", 64), ("oh1", JT), ("bb2", 64), ("oh2", JT)]:
    _CREG[_name] = (_c, _c + _w)
    _c += _w
_CW = _c


def _kmax_h(t: int) -> int:
    if t == 0:
        return 9
    return int(np.floor(np.log2(JT * t))) + 1


def _bcast(ap, n_dist: int, rep: int):
    return ap.unsqueeze(2).broadcast_to([ap.shape[0], n_dist, rep])


def _emit_heap_mms(nc, entries, w_of, m, kw, t0_open, after_k0=None, on_last=None):
    """k-step heap matmuls.  entries: (ps, r, c, t, h) x8 (two batches x 4
    half-space tiles).  t0_open: tile-0's k=0 opens its accumulation group
    (True when no bias matmul was emitted first).  Tile 0 always closes its
    own group (stop on its last matmul)."""
    kmaxes = [_kmax_h(t) for _, _, _, t, _ in entries]
    for k in range(0, max(kmaxes) + 1):
        deferred = []
        if k == 1 and after_k0 is not None:
            after_k0()
        for (ps, r, c, t, h), kmax in zip(entries, kmaxes):
            if k > kmax:
                continue
            c0 = t * JT
            hs = h[r : r + kw, :]
            om = ps[c : c + m, :]
            last = k == kmax
            if k == 0:
                nc.tensor.matmul(
                    om, w_of(r, k), hs[:, c0 : c0 + JT],
                    start=(t != 0) or t0_open, stop=last and t != 0,
                    tile_position=(r, c), skip_group_check=True,
                )
            elif t == 0:
                half = 1 << (k - 1)
                blk = 1 << k
                nc.tensor.matmul(
                    ps[c : c + m, half:blk],
                    w_of(r, k),
                    hs[:, 0:1].broadcast_to([kw, half]),
                    start=False, stop=last and blk >= JT,
                    tile_position=(r, c), skip_group_check=True,
                )
                if blk < JT:
                    nd = (JT >> k) - 1
                    deferred.append((ps, r, c, m, kw, hs, nd, blk, last, k))
            elif (JT >> k) >= 1:
                nd = JT >> k
                a0 = c0 >> k
                nc.tensor.matmul(
                    om, w_of(r, k), _bcast(hs[:, a0 : a0 + nd], nd, 1 << k),
                    start=False, stop=last,
                    tile_position=(r, c), skip_group_check=True,
                )
            else:
                a = c0 >> k
                nc.tensor.matmul(
                    om, w_of(r, k), hs[:, a : a + 1].broadcast_to([kw, JT]),
                    start=False, stop=last,
                    tile_position=(r, c), skip_group_check=True,
                )
        for ps, r, c, m2, kw2, hs2, nd, blk, last2, kk in deferred:
            nc.tensor.matmul(
                ps[c : c + m2, blk:JT],
                w_of(r, kk),
                _bcast(hs2[:, 1 : 1 + nd], nd, blk),
                start=False, stop=last2,
                tile_position=(r, c), skip_group_check=True,
            )
        if on_last is not None:
            for i, kmax in enumerate(kmaxes):
                if kmax == k:
                    on_last(i)


# Per batch-pair tile->quad-slot assignment.  Batch 0 of the pair streams
# from SBUF rows 0-63 (r=0), batch 1 from rows 64-127 (r=64).  Output col
# position alternates so each quad generation covers all 4 PE quadrants.
# (bi, t, r, c): bi = index within pair.
_SLOTS = [
    # quad A: tiles 0,1 of both batches (emission alternates row groups so
    # each LDWEIGHTS can overlap the other row-half's in-flight stream)
    [(0, 0, 0, 0), (1, 0, 64, 0), (0, 1, 0, 64), (1, 1, 64, 64)],
    # quad B: tiles 2,3
    [(0, 2, 0, 0), (1, 2, 64, 0), (0, 3, 0, 64), (1, 3, 64, 64)],
]
# L3 (m=2): all 8 tiles in ONE generation — 4 col strips x 2 row groups
# gives 8 disjoint PE rectangles (8 concurrent streams, 8 PSUM banks).
_SLOTS3 = [[
    (0, 0, 0, 0), (1, 0, 64, 0), (0, 1, 0, 32), (1, 1, 64, 32),
    (0, 2, 0, 64), (1, 2, 64, 64), (0, 3, 0, 96), (1, 3, 64, 96),
]]


def build_program(nb: int) -> bass.Bass:
    nc = bacc.Bacc()
    sxd = nc.declare_dram_parameter("sxd", [nb, 24, HUNITS], BF16, isOutput=False)
    cst = nc.declare_dram_parameter("cst", [128, _CW], BF16, isOutput=False)
    cstf = nc.declare_dram_parameter("cstf", [128, 4], F32, isOutput=False)
    outh = nc.declare_dram_parameter("outh", [nb, 2, HUNITS], F32, isOutput=True)

    relu = mybir.ActivationFunctionType.Relu
    add = mybir.AluOpType.add
    amax = mybir.AluOpType.max

    with tile.TileContext(nc) as tc:
        with (
            tc.tile_pool(name="const", bufs=1) as cp,
            tc.tile_pool(name="sx", bufs=3) as sxp,
            tc.tile_pool(name="r1", bufs=3) as r1p,
            tc.tile_pool(name="rh2", bufs=3) as r2p,
            tc.tile_pool(name="st", bufs=2) as stp,
            tc.tile_pool(name="ps", bufs=8, space="PSUM") as psp,
        ):
            cst_sb = cp.tile([128, _CW], BF16)
            nc.sync.dma_start(out=cst_sb[:], in_=cst[:, :])
            cstf_sb = cp.tile([128, 4], F32)
            nc.sync.dma_start(out=cstf_sb[:], in_=cstf[:, :])

            def _reg(name, rows):
                lo, hi = _CREG[name]

                def f(r):
                    return cst_sb[r : r + rows, lo:hi]

                return f

            w1s = _reg("w1s", 24)
            w2t = _reg("w2t", 64)
            w3t = _reg("w3t", 64)
            bb1 = _reg("bb1", 12)
            oh1 = _reg("oh1", 12)
            bb2 = _reg("bb2", 13)
            oh2 = _reg("oh2", 13)

            def w2_of(r, k):
                return w2t(r)[:, 64 * k : 64 * (k + 1)]

            def w3_of(r, k):
                return w3t(r)[:, 2 * k : 2 * (k + 1)]

            def bv(i, p):
                # bias vector APs (f32), at partition base p (0 or 64)
                return cstf_sb[p : p + 64, i : i + 1]

            def relu_copies(nc, quads, dst, bias_idx):
                """PSUM->SBUF relu copies for one layer of a batch pair.
                dst: single [128, 2048] tile, rows 0-63 = batch 0 of the
                pair, 64-127 = batch 1.  Copies go straight to the batch's
                half (cross-partition-base PSUM reads are legal)."""
                eng = [0, 1, 0, 1, 1, 0, 1, 0]
                for i, (ps, bi, t, r, c) in enumerate(quads):
                    want = 64 * bi
                    cs = slice(t * JT, (t + 1) * JT)
                    b_idx = bias_idx[t]
                    src = ps[c : c + 64, :]
                    out = dst[want : want + 64, cs]
                    if b_idx is None:
                        if eng[i] == 0:
                            nc.scalar.activation(out, src, relu)
                        else:
                            nc.vector.tensor_scalar(out, src, 0.0, None, amax)
                    else:
                        if eng[i] == 0:
                            nc.scalar.activation(out, src, relu, bias=bv(b_idx, c))
                        else:
                            nc.vector.tensor_scalar(out, src, bv(b_idx, c), 0.0,
                                                    add, amax)

            def emit_l1(bp):
                b0 = 2 * bp
                sx_sb = sxp.tile([128, HUNITS], BF16, name="sx_sb")
                nc.sync.dma_start(out=sx_sb[0:24, :], in_=sxd[b0, :, :])
                nc.sync.dma_start(out=sx_sb[64:88, :], in_=sxd[b0 + 1, :, :])
                r1 = r1p.tile([128, HUNITS], BF16, name="r1")
                for grp in _SLOTS:
                    quads = []
                    for bi, t, r, c in grp:
                        pq = psp.tile([128, JT], F32, tag="ps", name="pq")
                        quads.append((pq, bi, t, r, c))
                    for ps, bi, t, r, c in quads:
                        if t == 0:
                            nc.tensor.matmul(
                                ps[c : c + 64, :], bb1(r), oh1(r),
                                start=True, stop=False,
                                tile_position=(r, c), skip_group_check=True,
                            )
                    for ps, bi, t, r, c in quads:
                        nc.tensor.matmul(
                            ps[c : c + 64, :], w1s(r),
                            sx_sb[r : r + 24, t * JT : (t + 1) * JT],
                            start=t != 0, stop=True,
                            tile_position=(r, c), skip_group_check=True,
                        )
                    relu_copies(nc, quads, r1, [None, 0, 1, 1])
                return r1

            def emit_l2(bp, r1):
                rh2 = r2p.tile([128, HUNITS], BF16, name="rh2")
                for grp in _SLOTS:
                    quads = []
                    for bi, t, r, c in grp:
                        pq = psp.tile([128, JT], F32, tag="ps", name="pq")
                        quads.append((pq, bi, t, r, c))
                    for ps, bi, t, r, c in quads:
                        if t == 0:
                            nc.tensor.matmul(
                                ps[c : c + 64, :], bb2(r), oh2(r),
                                start=True, stop=False,
                                tile_position=(r, c), skip_group_check=True,
                            )
                    entries = [(ps, r, c, t, r1[:]) for ps, bi, t, r, c in quads]
                    _emit_heap_mms(nc, entries, w2_of, 64, 64, t0_open=False)
                    relu_copies(nc, quads, rh2, [None, 2, 3, 3])
                return rh2

            def emit_l3(bp, rh2):
                b0 = 2 * bp
                st = stp.tile([128, 2 * HUNITS], F32, name="st")
                for grp in _SLOTS3:
                    quads = []
                    for bi, t, r, c in grp:
                        pq = psp.tile([128, JT], F32, tag="ps", name="pq")
                        quads.append((pq, bi, t, r, c))
                    entries = [(ps, r, c, t, rh2[:]) for ps, bi, t, r, c in quads]

                    def _stage(i):
                        ps, bi, t, r, c = quads[i]
                        o = bi * HUNITS + t * JT
                        nc.any.tensor_copy(st[c : c + 2, o : o + JT],
                                           ps[c : c + 2, :])
                        nc.sync.dma_start(
                            out=outh[b0 + bi, :, t * JT : (t + 1) * JT],
                            in_=st[c : c + 2, o : o + JT],
                        )

                    _emit_heap_mms(nc, entries, w3_of, 2, 64, t0_open=True,
                                   on_last=_stage)

            # software pipeline over batch pairs: L2(p) / L1(p+1) / L3(p) —
            # each block's inputs are produced by copies that overlap the
            # previous block's matmuls, so the PE never hits a layer barrier.
            npairs = nb // 2
            r1_cur = emit_l1(0)
            rh2_prev = None
            for p in range(npairs):
                rh2_cur = emit_l2(p, r1_cur)
                if p + 1 < npairs:
                    r1_cur = emit_l1(p + 1)
                if rh2_prev is not None:
                    emit_l3(p - 1, rh2_prev)
                rh2_prev = rh2_cur
            emit_l3(npairs - 1, rh2_prev)
    nc.compile()
    return nc


def _prep_host(x, W1, b1, W2, b2, W3, b3):
    npbf = np.dtype(mybir.dt.np(BF16))
    lam = np.full(HUNITS, -1, np.int64)
    lam[1:] = np.floor(np.log2(np.arange(1, HUNITS))).astype(np.int64)

    B = x.shape[0]
    xt = np.ascontiguousarray(x.transpose(0, 2, 1))  # [B, 2, 4096]
    SX = np.zeros((B, 24, HUNITS), np.float32)
    for m in range(12):
        nd = HUNITS >> m
        rep = np.repeat(xt[:, :, :nd], 1 << m, axis=2)[:, :, :HUNITS]
        lo = 1 << (m - 1) if m >= 1 else 0
        if lo:
            rep[:, :, :lo] = 0.0
        SX[:, 2 * m : 2 * m + 2, :] = rep

    cum1 = np.cumsum(b1[:12], axis=0)
    cum2 = np.cumsum(b2, axis=0)
    w1s = np.zeros((24, 64), np.float32)
    for m in range(12):
        w1s[2 * m : 2 * m + 2, :] = W1[m].T
    w2t = W2[:12].transpose(2, 0, 1).reshape(64, 12 * 64)
    w3t = W3[:12].transpose(2, 0, 1).reshape(64, 12 * 2)

    L1lev = np.zeros(JT, np.int64)
    L1lev[1:] = lam[1:JT] + 1
    oh1 = np.zeros((12, JT), np.float32)
    oh1[L1lev, np.arange(JT)] = 1.0
    bb1 = cum1

    seg = np.zeros(JT, np.int64)
    seg[1:] = lam[1:JT] + 1
    oh2 = np.zeros((13, JT), np.float32)
    oh2[seg, np.arange(JT)] = 1.0
    bb2 = np.zeros((13, 64), np.float32)
    bb2[0] = cum2[1]
    for s in range(12):
        bb2[s + 1] = cum2[min(s + 2, 12)]

    cstm = np.zeros((128, _CW), np.float32)
    for name, a in [("w2t", w2t), ("w3t", w3t), ("w1s", w1s),
                    ("bb1", bb1), ("oh1", oh1), ("bb2", bb2), ("oh2", oh2)]:
        lo, hi = _CREG[name]
        cstm[: a.shape[0], lo:hi] = a
        cstm[64 : 64 + a.shape[0], lo:hi] = a

    cstfm = np.zeros((128, 4), np.float32)
    for half in (0, 64):
        cstfm[half : half + 64, 0] = cum1[10]
        cstfm[half : half + 64, 1] = cum1[11]
        cstfm[half : half + 64, 2] = cum2[11]
        cstfm[half : half + 64, 3] = cum2[12]

    common = {"cst": cstm.astype(npbf), "cstf": cstfm}
    nb = B // NCORES
    in_maps = []
    for c in range(NCORES):
        m = dict(common)
        m["sxd"] = SX[c * nb : (c + 1) * nb].astype(npbf)
        in_maps.append(m)

    rh2_0 = np.maximum(b2[0], 0.0)
    cum3 = np.cumsum(b3, axis=0)
    hb = np.zeros((HUNITS, 2), np.float32)
    hb[0] = cum3[1] + W3[1] @ rh2_0
    kk = np.minimum(lam[1:] + 2, 12)
    hb[1:] = cum3[kk] + np.einsum("nij,j->ni", W3[kk], rh2_0)
    out0 = W3[0] @ rh2_0 + b3[0]
    return in_maps, hb, out0


def kernel(x, W1, b1, W2, b2, W3, b3, _trace=False, _mode=None):
    nb = x.shape[0] // NCORES
    nc = build_program(nb)
    in_maps, hb, out0 = _prep_host(x, W1, b1, W2, b2, W3, b3)
    res = run_bass_kernel_spmd(
        nc, in_maps, core_ids=list(range(NCORES)), trace=_trace
    )
    outs = []
    for r in res.results:
        u0 = r["outh"].astype(np.float32)        # [nb, 2, 2048]
        u0 = u0 + hb.T[None]
        full = np.empty((u0.shape[0], UNITS, 2), np.float32)
        full[:, 0, :] = out0[None]
        full[:, 1, :] = u0[:, :, 0]
        full[:, 2:, :] = np.repeat(u0[:, :, 1:], 2, axis=2).transpose(0, 2, 1)
        outs.append(full)
    out = np.concatenate(outs, 0)
    if _trace:
        kernel.last_exec_time_ns = res.exec_time_ns
        kernel.last_results = res
    return out


kernel.last_exec_time_ns = None
kernel.last_results = None
MODE = "bf16-halfspace-pair"
